# revision 5
# baseline (speedup 1.0000x reference)
"""Trainium2 Bass kernel for nn_DS_Fusion_56495999811926 (dense_cnn).

Strategy: pure data parallelism — batch 16 sharded 2-per-core across 8
NeuronCores, weights replicated, no collectives.

v2 redesign vs baseline:
  - Both streams stacked into one [96, TN] tile: rb1/rb2/q/v/cf each become a
    single (block-diagonal) matmul and single ACT/DVE epilogue.
  - e2 conv fused into the k1/k2 convs on host (kk = (K@E2)@h + const), so
    the per-iter e2 matmul + la activation disappear; h (gelu output) is the
    iteration state.
  - Mixed precision around the 4-cyc/row fp32 PE mode:
      split3 (exact, 3 passes @ f32r 1 cyc/row): rb1, q, v, cf — rhs is the
        shared X tile, split once per iter into (hi = X & 0xFFFFF000, lo)
        on DVE; weights split on host.
      f32 native (exact): rb2, e1, e1k0 (cheap K, rhs produced on ACT).
      wsplit2 (2 passes, host-split weights, rhs rounded once by HW): fused
        k1/k2 convs, final e2.
      f32r single-pass: attention plumbing (ones/sw/perm/negI8/attexp/
        sum4/bc28) — 0/1 matrices exact, only the rhs rounds to 12-bit
        mantissa.
  - TN=512: every PSUM tile is exactly one bank -> 8 tiles in flight.
  - ones+sw merged into one [112,16] weight so max-iters cost 2 passes.
"""
import numpy as np

EPS = 1e-5

B, C, H, W = 16, 48, 128, 128
N_CORES = 8
B_LOC = B // N_CORES
HW = H * W
TN = 512

_prog_cache = {}

# gapped m-block row ranges in the 112-row layout
_BLK = [(0, 24), (24, 48), (64, 88), (88, 112)]


# ---------------------------------------------------------------- host math
def _round12(a):
    m, e = np.frexp(np.asarray(a, np.float64))
    return np.ldexp(np.round(m * 4096.0) / 4096.0, e)


def _split12(w):
    hi = _round12(w)
    lo = np.asarray(w, np.float64) - hi
    return hi.astype(np.float32), lo.astype(np.float32)


def fold_params(inp):
    f32, f64 = np.float32, np.float64
    P = {}

    def bn_sc(pref):
        s = np.asarray(inp[pref + '_g'], f64) / np.sqrt(
            np.asarray(inp[pref + '_v'], f64) + EPS)
        t = np.asarray(inp[pref + '_b'], f64) - np.asarray(inp[pref + '_m'],
                                                           f64) * s
        return s, t

    # rb1 stacked block-diag [96,48]
    s_rb, t_rb = bn_sc('rb_bn')
    rb1T = (s_rb[:, None] * np.asarray(inp['rb_w1'], f64)).T        # [48,24]
    rb1s = np.zeros((96, 48), f64)
    rb1s[0:48, 0:24] = rb1T
    rb1s[48:96, 24:48] = rb1T
    P['rb1s_hi'], P['rb1s_lo'] = _split12(rb1s)
    b1 = s_rb * np.asarray(inp['rb_b1'], f64) + t_rb
    P['b_rb1s'] = np.concatenate([b1, b1])[:, None].astype(f32)     # [48,1]

    # rb2 stacked block-diag [48,96], f32 native
    s_bn, t_bn = bn_sc('bn')
    rb2T = (s_bn[:, None] * np.asarray(inp['rb_w2'], f64)).T        # [24,48]
    rb2s = np.zeros((48, 96), f64)
    rb2s[0:24, 0:48] = rb2T
    rb2s[24:48, 48:96] = rb2T
    P['rb2s'] = rb2s.astype(f32)
    b2 = s_bn * np.asarray(inp['rb_b2'], f64) + t_bn
    P['b_rb2s'] = np.concatenate([b2, b2])[:, None].astype(f32)     # [96,1]
    P['sxv96'] = np.concatenate([s_bn, s_bn])[:, None].astype(f32)

    def gap_bias(b48):
        g = np.zeros((112, 1), f64)
        g[0:48, 0] = b48
        g[64:112, 0] = b48
        return g.astype(f32)

    # q/v stacked into the gapped 112 layout: [96,112]
    s_q, t_q = bn_sc('q_bn')
    qw = (s_q[:, None] * np.asarray(inp['q_w'], f64)).T             # [48,48]
    qb = s_q * np.asarray(inp['q_b'], f64) + t_q
    qs = np.zeros((96, 112), f64)
    qs[0:48, 0:48] = qw
    qs[48:96, 64:112] = qw
    P['qs_hi'], P['qs_lo'] = _split12(qs)
    P['bq_g'] = gap_bias(qb)
    s_v, t_v = bn_sc('v_bn')
    vw = (s_v[:, None] * np.asarray(inp['v_w'], f64)).T
    vb = s_v * np.asarray(inp['v_b'], f64) + t_v
    vs = np.zeros((96, 112), f64)
    vs[0:48, 0:48] = vw
    vs[48:96, 64:112] = vw
    P['vs_hi'], P['vs_lo'] = _split12(vs)
    P['bv_g'] = gap_bias(vb)

    # cross-fusion stacked [96,48]
    s_cf, t_cf = bn_sc('cf_bn')
    cw = (s_cf[:, None] * np.asarray(inp['cf_w'], f64)).T           # [96,48]
    P['cfs_hi'], P['cfs_lo'] = _split12(cw)
    P['b_cf'] = (s_cf * np.asarray(inp['cf_b'], f64) + t_cf)[:, None
                                                             ].astype(f32)

    # fused (k_p . e2) convs: kk_p = (K_p@E2)@h + (K_p@be2 + bk_p)
    e2w = np.asarray(inp['emb_w2'], f64)                            # [48,24]
    be2 = np.asarray(inp['emb_b2'], f64)                            # [48]
    for i, pref in enumerate(('k1', 'k2')):
        s_k, t_k = bn_sc(pref + '_bn')
        kwp = s_k[:, None] * np.asarray(inp[pref + '_w'], f64)      # [24,48]
        bkp = s_k * np.asarray(inp[pref + '_b'], f64) + t_k
        fw = kwp @ e2w                                              # [24,24]
        fb = kwp @ be2 + bkp
        kg = np.zeros((24, 112), f64)
        bg = np.zeros((112, 1), f64)
        for m in range(4):
            lo, hi = _BLK[m]
            kg[:, lo:hi] = fw.T
            bg[lo:hi, 0] = fb
        P[f'kf{i + 1}_hi'], P[f'kf{i + 1}_lo'] = _split12(kg)
        P[f'bkf{i + 1}'] = bg.astype(f32)

    # e1 (f32 native) + k0 path + final e2 (wsplit2)
    w1 = np.asarray(inp['emb_w1'], f64)                             # [24,48]
    e1a = np.zeros((112, 24), f64)
    e1b = np.zeros((112, 24), f64)
    for m in range(4):
        lo, hi = _BLK[m]
        e1a[lo:hi] = w1[:, :24].T
        e1b[lo:hi] = w1[:, 24:].T
    P['e1aT'] = e1a.astype(f32)
    P['e1bT'] = e1b.astype(f32)
    P['e1Tk0'] = w1.T.astype(f32)                                   # [48,24]
    P['b_e1'] = np.asarray(inp['emb_b1'], f64)[:, None].astype(f32)
    P['e2hT_hi'], P['e2hT_lo'] = _split12(e2w.T)                    # [24,48]
    P['b_e2'] = be2[:, None].astype(f32)

    # attention constants: merged ones+sw [112,40] (sw block at col 32 so
    # the PSUM read of the sw rows starts at partition 32), attexp [8,112]
    for p in range(2):
        o = np.zeros((112, 40), f32)
        for m in range(4):
            lo, hi = _BLK[m]
            o[lo:hi, 4 * p + m] = 1.0
            losw, hisw = _BLK[m ^ 1]
            o[losw:hisw, 32 + 4 * p + m] = 1.0
        P[f'ones16_{p + 1}'] = o
        ae = np.zeros((8, 112), f32)
        for m in range(4):
            lo, hi = _BLK[m]
            ae[4 * p + m, lo:hi] = 1.0
        P[f'attexp{p + 1}T'] = ae
    sum4 = np.zeros((8, 2), f32)
    bc28 = np.zeros((2, 8), f32)
    for p in range(2):
        sum4[4 * p:4 * (p + 1), p] = 1.0
        bc28[p, 4 * p:4 * (p + 1)] = 1.0
    P['sum4T'] = sum4
    P['bc28T'] = bc28
    perm8 = np.zeros((8, 8), f32)
    for c, k in enumerate([2, 3, 0, 1, 6, 7, 4, 5]):
        perm8[k, c] = 1.0
    P['perm8T'] = perm8
    P['negI8'] = (-np.eye(8)).astype(f32)
    return P


# ---------------------------------------------------------------- program
def build_program(b_loc=B_LOC, hw=HW, tn=TN, use_f32r=True,
                  max_ks=(2, 3), repeat=1):
    import concourse.bacc as bacc
    import concourse.mybir as mybir
    from concourse import tile
    from concourse.dve_ops import (RECIP_APPROX_FAST_CONSTS as _RC,
                                   RECIPROCAL_APPROX_FAST as _RF)

    f32 = mybir.dt.float32
    f32r = mybir.dt.float32r
    i32 = mybir.dt.int32
    A = mybir.ActivationFunctionType
    OP = mybir.AluOpType
    NH = tn // 512

    nc = bacc.Bacc(None, target_bir_lowering=False)

    # name -> (shape, dtype); f32r for matmul operands, f32 for exact/bias
    wshapes = dict(
        rb1s_hi=((96, 48), f32r), rb1s_lo=((96, 48), f32r),
        b_rb1s=((48, 1), f32),
        rb2s=((48, 96), f32), b_rb2s=((96, 1), f32), sxv96=((96, 1), f32),
        qs_hi=((96, 112), f32r), qs_lo=((96, 112), f32r),
        bq_g=((112, 1), f32),
        vs_hi=((96, 112), f32r), vs_lo=((96, 112), f32r),
        bv_g=((112, 1), f32),
        cfs_hi=((96, 48), f32r), cfs_lo=((96, 48), f32r),
        b_cf=((48, 1), f32),
        kf1_hi=((24, 112), f32r), kf1_lo=((24, 112), f32r),
        bkf1=((112, 1), f32),
        kf2_hi=((24, 112), f32r), kf2_lo=((24, 112), f32r),
        bkf2=((112, 1), f32),
        e1aT=((112, 24), f32), e1bT=((112, 24), f32), b_e1=((24, 1), f32),
        e1Tk0=((48, 24), f32),
        e2hT_hi=((24, 48), f32r), e2hT_lo=((24, 48), f32r),
        b_e2=((48, 1), f32),
        ones16_1=((112, 40), f32r), ones16_2=((112, 40), f32r),
        attexp1T=((8, 112), f32r), attexp2T=((8, 112), f32r),
        sum4T=((8, 2), f32r), bc28T=((2, 8), f32r),
        perm8T=((8, 8), f32r), negI8=((8, 8), f32r),
    )

    dram = {}
    for name, (shp, dt) in wshapes.items():
        dram[name] = nc.declare_dram_parameter(name, list(shp), dt,
                                               isOutput=False)
    x0_d = nc.declare_dram_parameter("x0", [b_loc, 48, hw], f32,
                                     isOutput=False)
    x1_d = nc.declare_dram_parameter("x1", [b_loc, 48, hw], f32,
                                     isOutput=False)
    out_d = nc.declare_dram_parameter("out", [b_loc, 48, hw], f32,
                                      isOutput=True)

    nchunk = b_loc * hw // tn
    per_img = hw // tn

    with tile.TileContext(nc) as tc:
        with (tc.tile_pool(name="wp", bufs=1) as wp,
              tc.tile_pool(name="xp", bufs=3) as xp,
              tc.tile_pool(name="hp", bufs=3) as hp,
              tc.tile_pool(name="sp", bufs=2) as sp,
              tc.tile_pool(name="up", bufs=3) as up,
              tc.tile_pool(name="pp", bufs=8, space="PSUM") as pp):
            WT = {}
            for name, (shp, dt) in wshapes.items():
                t = wp.tile(list(shp), dt, name=f"w_{name}")
                nc.sync.dma_start(out=t[:, :], in_=dram[name][:, :])
                WT[name] = t

            def mm(ps, lhsT, rhs, start, stop):
                for hh in range(NH):
                    sl = slice(512 * hh, 512 * (hh + 1))
                    nc.tensor.matmul(ps[:, sl], lhsT, rhs[:, sl],
                                     start=start, stop=stop,
                                     skip_group_check=True)

            def mm3(ps, whi, wlo, rhi, rlo):
                mm(ps, whi, rhi, True, False)
                mm(ps, whi, rlo, False, False)
                mm(ps, wlo, rhi, False, True)

            def psum(rows, name):
                return pp.tile([rows, tn], f32, tag="ps", name=name,
                               padded_shape=[128, tn])

            def split_lo(x, xhi, ci, k, rows=96):
                xlo = xp.tile([rows, tn], f32r, tag="xlo",
                              name=f"xlo_{ci}_{k}")
                nc.vector.tensor_tensor(out=xlo[:, :], in0=x[:, :],
                                        in1=xhi[:, :], op=OP.subtract)
                return xlo

            from contextlib import nullcontext
            rep_ctx = tc.For_i(0, repeat, 1) if repeat > 1 else nullcontext()
            with rep_ctx:
              for ci in range(nchunk):
                bimg, off = ci // per_img, (ci % per_img) * tn
                X = xp.tile([96, tn], f32, tag="x", name=f"x_{ci}")
                nc.sync.dma_start(out=X[0:48, :],
                                  in_=x0_d[bimg, :, off:off + tn])
                nc.sync.dma_start(out=X[48:96, :],
                                  in_=x1_d[bimg, :, off:off + tn])
                Xhi = xp.tile([96, tn], f32r, tag="xhi", name=f"xhi_{ci}")
                nc.scalar.activation(Xhi[:, :], X[:, :], A.Identity)
                Xlo = split_lo(X, Xhi, ci, "in")
                h = None
                for k in range(4):
                    # --- residual refinement (stacked streams) ---
                    ps_r = psum(48, f"psr_{ci}_{k}")
                    mm3(ps_r, WT['rb1s_hi'][:, :], WT['rb1s_lo'][:, :],
                        Xhi, Xlo)
                    r_ = hp.tile([48, tn], f32, tag="r", name=f"r_{ci}_{k}")
                    nc.scalar.activation(r_[:, :], ps_r[:, :], A.Relu,
                                         bias=WT['b_rb1s'][:, 0:1])
                    ps_x = psum(96, f"psx_{ci}_{k}")
                    mm(ps_x, WT['rb2s'][:, :], r_, True, True)
                    sx = sp.tile([96, tn], f32, tag="sx", name=f"sx_{ci}_{k}")
                    nc.vector.scalar_tensor_tensor(
                        sx[:, :], X[:, :], WT['sxv96'][:, 0:1],
                        ps_x[:, :], op0=OP.mult, op1=OP.add)
                    X = xp.tile([96, tn], f32, tag="x", name=f"x_{ci}_{k}")
                    nc.scalar.activation(X[:, :], sx[:, :], A.Relu,
                                         bias=WT['b_rb2s'][:, 0:1])
                    Xhi = xp.tile([96, tn], f32r, tag="xhi",
                                  name=f"xhi_{ci}_{k}")
                    nc.scalar.activation(Xhi[:, :], sx[:, :], A.Relu,
                                         bias=WT['b_rb2s'][:, 0:1])
                    Xlo = split_lo(X, Xhi, ci, k)
                    # --- q/v convs into gapped 112 layout ---
                    ps_q = psum(112, f"psq_{ci}_{k}")
                    mm3(ps_q, WT['qs_hi'][:, :], WT['qs_lo'][:, :], Xhi, Xlo)
                    qall = up.tile([112, tn], f32, tag="qall",
                                   name=f"q_{ci}_{k}")
                    nc.scalar.activation(qall[:, :], ps_q[:, :], A.Identity,
                                         bias=WT['bq_g'][:, 0:1])
                    ps_v = psum(112, f"psv_{ci}_{k}")
                    mm3(ps_v, WT['vs_hi'][:, :], WT['vs_lo'][:, :], Xhi, Xlo)
                    vall = up.tile([112, tn], f32, tag="vall",
                                   name=f"v_{ci}_{k}")
                    nc.scalar.activation(vall[:, :], ps_v[:, :], A.Identity,
                                         bias=WT['bv_g'][:, 0:1])
                    # --- k0: cross fusion -> h0 ---
                    if k == 0:
                        ps_cf = psum(48, f"pscf_{ci}")
                        mm3(ps_cf, WT['cfs_hi'][:, :], WT['cfs_lo'][:, :],
                            Xhi, Xlo)
                        la0 = sp.tile([48, tn], f32, tag="la0",
                                      name=f"la0_{ci}")
                        nc.scalar.activation(la0[:, :], ps_cf[:, :], A.Relu,
                                             bias=WT['b_cf'][:, 0:1])
                        ps_h0 = psum(24, f"psh0_{ci}")
                        mm(ps_h0, WT['e1Tk0'][:, :], la0, True, True)
                        h = hp.tile([24, tn], f32r, tag="h", name=f"h0_{ci}")
                        nc.scalar.activation(h[:, :], ps_h0[:, :], A.Gelu,
                                             bias=WT['b_e1'][:, 0:1])
                    # --- attention logits ---
                    ts_ = []
                    for p in range(2):
                        ps_kk = psum(112, f"pskk{p}_{ci}_{k}")
                        mm(ps_kk, WT[f'kf{p + 1}_hi'][:, :], h, True, False)
                        mm(ps_kk, WT[f'kf{p + 1}_lo'][:, :], h, False, True)
                        t_ = hp.tile([112, tn], f32r, tag="t",
                                     name=f"t{p}_{ci}_{k}")
                        nc.vector.scalar_tensor_tensor(
                            t_[:, :], ps_kk[:, :], WT[f'bkf{p + 1}'][:, 0:1],
                            qall[:, :], op0=OP.add, op1=OP.mult)
                        ts_.append(t_)
                    do_max = k in max_ks
                    mrows = 40 if do_max else 8
                    ps_ls = psum(mrows, f"psls_{ci}_{k}")
                    for p in range(2):
                        mm(ps_ls, WT[f'ones16_{p + 1}'][:, 0:mrows], ts_[p],
                           p == 0, p == 1 and not do_max)
                    if do_max:
                        sw_sb = sp.tile([8, tn], f32, tag="swsb",
                                        name=f"swsb_{ci}_{k}")
                        nc.scalar.activation(sw_sb[:, :], ps_ls[32:40, :],
                                             A.Identity)
                        mx1 = sp.tile([8, tn], f32r, tag="mx1",
                                      name=f"mx1_{ci}_{k}")
                        nc.vector.tensor_tensor(out=mx1[:, :],
                                                in0=ps_ls[0:8, :],
                                                in1=sw_sb[:, :], op=OP.max)
                        ps_pm = psum(8, f"pspm_{ci}_{k}")
                        mm(ps_pm, WT['perm8T'][:, :], mx1, True, True)
                        mxf = sp.tile([8, tn], f32r, tag="mxf",
                                      name=f"mxf_{ci}_{k}")
                        nc.vector.tensor_tensor(out=mxf[:, :], in0=mx1[:, :],
                                                in1=ps_pm[:, :], op=OP.max)
                        mm(ps_ls[0:8, :], WT['negI8'][:, :], mxf, False, True)
                    # --- softmax + AV ---
                    e_ = sp.tile([8, tn], f32r, tag="e", name=f"e_{ci}_{k}")
                    nc.scalar.activation(e_[:, :], ps_ls[0:8, :], A.Exp)
                    ps_S = psum(2, f"psS_{ci}_{k}")
                    mm(ps_S, WT['sum4T'][:, :], e_, True, True)
                    rr = sp.tile([2, tn], f32r, tag="rr", name=f"rr_{ci}_{k}")
                    nc.vector._custom_dve(_RF, out=rr[:, :], in0=ps_S[:, :],
                                          s0=_RC["s0"], s1=_RC["s1"],
                                          imm2=_RC["imm2"])
                    ps_rbc = psum(8, f"psrbc_{ci}_{k}")
                    mm(ps_rbc, WT['bc28T'][:, :], rr, True, True)
                    att = sp.tile([8, tn], f32r, tag="att",
                                  name=f"att_{ci}_{k}")
                    nc.vector.tensor_tensor(out=att[:, :], in0=e_[:, :],
                                            in1=ps_rbc[:, :], op=OP.mult)
                    us = []
                    for p in range(2):
                        ps_ae = psum(112, f"psae{p}_{ci}_{k}")
                        mm(ps_ae, WT[f'attexp{p + 1}T'][:, :], att,
                           True, True)
                        u_ = up.tile([112, tn], f32, tag="u",
                                     name=f"u{p}_{ci}_{k}")
                        nc.vector.tensor_tensor(out=u_[:, :], in0=ps_ae[:, :],
                                                in1=vall[:, :], op=OP.mult)
                        us.append(u_)
                    ps_h = psum(24, f"psh_{ci}_{k}")
                    mm(ps_h, WT['e1aT'][:, :], us[0], True, False)
                    mm(ps_h, WT['e1bT'][:, :], us[1], False, True)
                    h = hp.tile([24, tn], f32r, tag="h", name=f"h_{ci}_{k}")
                    nc.scalar.activation(h[:, :], ps_h[:, :], A.Gelu,
                                         bias=WT['b_e1'][:, 0:1])
                # --- final e2 (wsplit2) + store ---
                ps_o = psum(48, f"pso_{ci}")
                mm(ps_o, WT['e2hT_hi'][:, :], h, True, False)
                mm(ps_o, WT['e2hT_lo'][:, :], h, False, True)
                out = sp.tile([48, tn], f32, tag="out", name=f"out_{ci}")
                nc.scalar.activation(out[:, :], ps_o[:, :], A.Identity,
                                     bias=WT['b_e2'][:, 0:1])
                nc.sync.dma_start(out=out_d[bimg, :, off:off + tn],
                                  in_=out[:, :])
    nc.compile()
    return nc


# ---------------------------------------------------------------- entry
def kernel(**inputs):
    from concourse.bass_utils import run_bass_kernel_spmd

    key = "full"
    if key not in _prog_cache:
        _prog_cache[key] = build_program()
    nc = _prog_cache[key]

    P = fold_params({k: np.asarray(v) for k, v in inputs.items()})
    x0 = np.asarray(inputs['x0'], np.float32).reshape(B, C, HW)
    x1 = np.asarray(inputs['x1'], np.float32).reshape(B, C, HW)
    in_maps = []
    for c in range(N_CORES):
        m = dict(P)
        m['x0'] = np.ascontiguousarray(x0[c * B_LOC:(c + 1) * B_LOC])
        m['x1'] = np.ascontiguousarray(x1[c * B_LOC:(c + 1) * B_LOC])
        in_maps.append(m)
    res = run_bass_kernel_spmd(nc, in_maps, list(range(N_CORES)))
    out = np.concatenate([res.results[c]['out'] for c in range(N_CORES)], 0)
    return out.reshape(B, C, H, W).astype(np.float32)


if __name__ == '__main__':
    import reference as R
    inputs = R.setup_inputs()
    expected = np.asarray(R.reference(**inputs))
    actual = kernel(**{k: np.asarray(v) for k, v in inputs.items()})
    denom = np.abs(expected).max()
    rel = np.abs(actual - expected).max() / denom
    print('rel err:', rel)


# revision 6
# speedup vs baseline: 35.3028x; 35.3028x over previous
"""Trainium2 Bass kernel for nn_DS_Fusion_56495999811926 (dense_cnn).

Strategy: pure data parallelism — batch 16 sharded 2-per-core across 8
NeuronCores, weights replicated, no collectives.

v2 redesign vs baseline:
  - Both streams stacked into one [96, TN] tile: rb1/rb2/q/v/cf each become a
    single (block-diagonal) matmul and single ACT/DVE epilogue.
  - e2 conv fused into the k1/k2 convs on host (kk = (K@E2)@h + const), so
    the per-iter e2 matmul + la activation disappear; h (gelu output) is the
    iteration state.
  - Mixed precision around the 4-cyc/row fp32 PE mode:
      split3 (exact, 3 passes @ f32r 1 cyc/row): rb1, q, v, cf — rhs is the
        shared X tile, split once per iter into (hi = X & 0xFFFFF000, lo)
        on DVE; weights split on host.
      f32 native (exact): rb2, e1, e1k0 (cheap K, rhs produced on ACT).
      wsplit2 (2 passes, host-split weights, rhs rounded once by HW): fused
        k1/k2 convs, final e2.
      f32r single-pass: attention plumbing (ones/sw/perm/negI8/attexp/
        sum4/bc28) — 0/1 matrices exact, only the rhs rounds to 12-bit
        mantissa.
  - TN=512: every PSUM tile is exactly one bank -> 8 tiles in flight.
  - ones+sw merged into one [112,16] weight so max-iters cost 2 passes.
"""
import numpy as np

EPS = 1e-5

B, C, H, W = 16, 48, 128, 128
N_CORES = 8
B_LOC = B // N_CORES
HW = H * W
TN = 512

_prog_cache = {}

# gapped m-block row ranges in the 112-row layout
_BLK = [(0, 24), (24, 48), (64, 88), (88, 112)]


# ---------------------------------------------------------------- host math
def _round12(a):
    m, e = np.frexp(np.asarray(a, np.float64))
    return np.ldexp(np.round(m * 4096.0) / 4096.0, e)


def _split12(w):
    hi = _round12(w)
    lo = np.asarray(w, np.float64) - hi
    return hi.astype(np.float32), lo.astype(np.float32)


def fold_params(inp):
    f32, f64 = np.float32, np.float64
    P = {}

    def bn_sc(pref):
        s = np.asarray(inp[pref + '_g'], f64) / np.sqrt(
            np.asarray(inp[pref + '_v'], f64) + EPS)
        t = np.asarray(inp[pref + '_b'], f64) - np.asarray(inp[pref + '_m'],
                                                           f64) * s
        return s, t

    # rb1 stacked block-diag [96,48]
    s_rb, t_rb = bn_sc('rb_bn')
    rb1T = (s_rb[:, None] * np.asarray(inp['rb_w1'], f64)).T        # [48,24]
    rb1s = np.zeros((96, 48), f64)
    rb1s[0:48, 0:24] = rb1T
    rb1s[48:96, 24:48] = rb1T
    P['rb1s_hi'], P['rb1s_lo'] = _split12(rb1s)
    b1 = s_rb * np.asarray(inp['rb_b1'], f64) + t_rb
    P['b_rb1s'] = np.concatenate([b1, b1])[:, None].astype(f32)     # [48,1]

    # rb2 stacked block-diag [48,96], f32 native
    s_bn, t_bn = bn_sc('bn')
    rb2T = (s_bn[:, None] * np.asarray(inp['rb_w2'], f64)).T        # [24,48]
    rb2s = np.zeros((48, 96), f64)
    rb2s[0:24, 0:48] = rb2T
    rb2s[24:48, 48:96] = rb2T
    P['rb2s'] = rb2s.astype(f32)
    b2 = s_bn * np.asarray(inp['rb_b2'], f64) + t_bn
    P['b_rb2s'] = np.concatenate([b2, b2])[:, None].astype(f32)     # [96,1]
    P['sxv96'] = np.concatenate([s_bn, s_bn])[:, None].astype(f32)

    def gap_bias(b48):
        g = np.zeros((112, 1), f64)
        g[0:48, 0] = b48
        g[64:112, 0] = b48
        return g.astype(f32)

    # q/v stacked into the gapped 112 layout: [96,112]
    s_q, t_q = bn_sc('q_bn')
    qw = (s_q[:, None] * np.asarray(inp['q_w'], f64)).T             # [48,48]
    qb = s_q * np.asarray(inp['q_b'], f64) + t_q
    qs = np.zeros((96, 112), f64)
    qs[0:48, 0:48] = qw
    qs[48:96, 64:112] = qw
    P['qs_hi'], P['qs_lo'] = _split12(qs)
    P['bq_g'] = gap_bias(qb)
    s_v, t_v = bn_sc('v_bn')
    vw = (s_v[:, None] * np.asarray(inp['v_w'], f64)).T
    vb = s_v * np.asarray(inp['v_b'], f64) + t_v
    vs = np.zeros((96, 112), f64)
    vs[0:48, 0:48] = vw
    vs[48:96, 64:112] = vw
    P['vs_hi'], P['vs_lo'] = _split12(vs)
    P['bv_g'] = gap_bias(vb)

    # cross-fusion stacked [96,48]
    s_cf, t_cf = bn_sc('cf_bn')
    cw = (s_cf[:, None] * np.asarray(inp['cf_w'], f64)).T           # [96,48]
    P['cfs_hi'], P['cfs_lo'] = _split12(cw)
    P['b_cf'] = (s_cf * np.asarray(inp['cf_b'], f64) + t_cf)[:, None
                                                             ].astype(f32)

    # fused (k_p . e2) convs: kk_p = (K_p@E2)@h + (K_p@be2 + bk_p)
    e2w = np.asarray(inp['emb_w2'], f64)                            # [48,24]
    be2 = np.asarray(inp['emb_b2'], f64)                            # [48]
    for i, pref in enumerate(('k1', 'k2')):
        s_k, t_k = bn_sc(pref + '_bn')
        kwp = s_k[:, None] * np.asarray(inp[pref + '_w'], f64)      # [24,48]
        bkp = s_k * np.asarray(inp[pref + '_b'], f64) + t_k
        fw = kwp @ e2w                                              # [24,24]
        fb = kwp @ be2 + bkp
        kg = np.zeros((24, 112), f64)
        bg = np.zeros((112, 1), f64)
        for m in range(4):
            lo, hi = _BLK[m]
            kg[:, lo:hi] = fw.T
            bg[lo:hi, 0] = fb
        P[f'kf{i + 1}_hi'], P[f'kf{i + 1}_lo'] = _split12(kg)
        P[f'bkf{i + 1}'] = bg.astype(f32)

    # e1 (f32 native) + k0 path + final e2 (wsplit2)
    w1 = np.asarray(inp['emb_w1'], f64)                             # [24,48]
    e1a = np.zeros((112, 24), f64)
    e1b = np.zeros((112, 24), f64)
    for m in range(4):
        lo, hi = _BLK[m]
        e1a[lo:hi] = w1[:, :24].T
        e1b[lo:hi] = w1[:, 24:].T
    P['e1aT'] = e1a.astype(f32)
    P['e1bT'] = e1b.astype(f32)
    P['e1Tk0'] = w1.T.astype(f32)                                   # [48,24]
    P['b_e1'] = np.asarray(inp['emb_b1'], f64)[:, None].astype(f32)
    P['e2hT_hi'], P['e2hT_lo'] = _split12(e2w.T)                    # [24,48]
    P['b_e2'] = be2[:, None].astype(f32)

    # attention constants: merged ones+sw [112,40] (sw block at col 32 so
    # the PSUM read of the sw rows starts at partition 32), attexp [8,112]
    for p in range(2):
        o = np.zeros((112, 40), f32)
        for m in range(4):
            lo, hi = _BLK[m]
            o[lo:hi, 4 * p + m] = 1.0
            losw, hisw = _BLK[m ^ 1]
            o[losw:hisw, 32 + 4 * p + m] = 1.0
        P[f'ones16_{p + 1}'] = o
        ae = np.zeros((8, 112), f32)
        for m in range(4):
            lo, hi = _BLK[m]
            ae[4 * p + m, lo:hi] = 1.0
        P[f'attexp{p + 1}T'] = ae
    sum4 = np.zeros((8, 2), f32)
    bc28 = np.zeros((2, 8), f32)
    for p in range(2):
        sum4[4 * p:4 * (p + 1), p] = 1.0
        bc28[p, 4 * p:4 * (p + 1)] = 1.0
    P['sum4T'] = sum4
    P['bc28T'] = bc28
    perm8 = np.zeros((8, 8), f32)
    for c, k in enumerate([2, 3, 0, 1, 6, 7, 4, 5]):
        perm8[k, c] = 1.0
    P['perm8T'] = perm8
    P['negI8'] = (-np.eye(8)).astype(f32)
    return P


# ---------------------------------------------------------------- program
def build_program(b_loc=B_LOC, hw=HW, tn=TN, use_f32r=True,
                  max_ks=(2, 3), repeat=1):
    import concourse.bacc as bacc
    import concourse.mybir as mybir
    from concourse import tile
    from concourse.dve_ops import (RECIP_APPROX_FAST_CONSTS as _RC,
                                   RECIPROCAL_APPROX_FAST as _RF)

    f32 = mybir.dt.float32
    f32r = mybir.dt.float32r
    i32 = mybir.dt.int32
    A = mybir.ActivationFunctionType
    OP = mybir.AluOpType
    NH = tn // 512

    nc = bacc.Bacc(None, target_bir_lowering=False)

    # name -> (shape, dtype); f32r for matmul operands, f32 for exact/bias
    wshapes = dict(
        rb1s_hi=((96, 48), f32r), rb1s_lo=((96, 48), f32r),
        b_rb1s=((48, 1), f32),
        rb2s=((48, 96), f32), b_rb2s=((96, 1), f32), sxv96=((96, 1), f32),
        qs_hi=((96, 112), f32r), qs_lo=((96, 112), f32r),
        bq_g=((112, 1), f32),
        vs_hi=((96, 112), f32r), vs_lo=((96, 112), f32r),
        bv_g=((112, 1), f32),
        cfs_hi=((96, 48), f32r), cfs_lo=((96, 48), f32r),
        b_cf=((48, 1), f32),
        kf1_hi=((24, 112), f32r), kf1_lo=((24, 112), f32r),
        bkf1=((112, 1), f32),
        kf2_hi=((24, 112), f32r), kf2_lo=((24, 112), f32r),
        bkf2=((112, 1), f32),
        e1aT=((112, 24), f32), e1bT=((112, 24), f32), b_e1=((24, 1), f32),
        e1Tk0=((48, 24), f32),
        e2hT_hi=((24, 48), f32r), e2hT_lo=((24, 48), f32r),
        b_e2=((48, 1), f32),
        ones16_1=((112, 40), f32r), ones16_2=((112, 40), f32r),
        attexp1T=((8, 112), f32r), attexp2T=((8, 112), f32r),
        sum4T=((8, 2), f32r), bc28T=((2, 8), f32r),
        perm8T=((8, 8), f32r), negI8=((8, 8), f32r),
    )

    dram = {}
    for name, (shp, dt) in wshapes.items():
        dram[name] = nc.declare_dram_parameter(name, list(shp), dt,
                                               isOutput=False)
    x0_d = nc.declare_dram_parameter("x0", [b_loc, 48, hw], f32,
                                     isOutput=False)
    x1_d = nc.declare_dram_parameter("x1", [b_loc, 48, hw], f32,
                                     isOutput=False)
    out_d = nc.declare_dram_parameter("out", [b_loc, 48, hw], f32,
                                      isOutput=True)

    nchunk = b_loc * hw // tn
    per_img = hw // tn

    with tile.TileContext(nc) as tc:
        with (tc.tile_pool(name="wp", bufs=1) as wp,
              tc.tile_pool(name="xp", bufs=5) as xp,
              tc.tile_pool(name="hp", bufs=5) as hp,
              tc.tile_pool(name="sp", bufs=4) as sp,
              tc.tile_pool(name="up", bufs=5) as up,
              tc.tile_pool(name="pp", bufs=8, space="PSUM") as pp):
            WT = {}
            for name, (shp, dt) in wshapes.items():
                t = wp.tile(list(shp), dt, name=f"w_{name}")
                nc.sync.dma_start(out=t[:, :], in_=dram[name][:, :])
                WT[name] = t

            def mm(ps, lhsT, rhs, start, stop):
                for hh in range(NH):
                    sl = slice(512 * hh, 512 * (hh + 1))
                    nc.tensor.matmul(ps[:, sl], lhsT, rhs[:, sl],
                                     start=start, stop=stop,
                                     skip_group_check=True)

            def mm3(ps, whi, wlo, rhi, rlo):
                mm(ps, whi, rhi, True, False)
                mm(ps, whi, rlo, False, False)
                mm(ps, wlo, rhi, False, True)

            def psum(rows, name):
                return pp.tile([rows, tn], f32, tag="ps", name=name,
                               padded_shape=[128, tn])

            def split_lo(x, xhi, ci, k, rows=96):
                xlo = xp.tile([rows, tn], f32r, tag="xlo",
                              name=f"xlo_{ci}_{k}")
                nc.vector.tensor_tensor(out=xlo[:, :], in0=x[:, :],
                                        in1=xhi[:, :], op=OP.subtract)
                return xlo

            from contextlib import nullcontext

            def new_state(ci):
                st = {'ci': ci}
                bimg, off = ci // per_img, (ci % per_img) * tn
                st['bimg'], st['off'] = bimg, off
                X = xp.tile([96, tn], f32, tag="x", name=f"x_{ci}")
                nc.sync.dma_start(out=X[0:48, :],
                                  in_=x0_d[bimg, :, off:off + tn])
                nc.sync.dma_start(out=X[48:96, :],
                                  in_=x1_d[bimg, :, off:off + tn])
                Xhi = xp.tile([96, tn], f32r, tag="xhi", name=f"xhi_{ci}")
                nc.gpsimd.tensor_copy(out=Xhi[:, :], in_=X[:, :])
                Xlo = xp.tile([96, tn], f32r, tag="xlo", name=f"xlo_{ci}")
                nc.vector.tensor_tensor(out=Xlo[:, :], in0=X[:, :],
                                        in1=Xhi[:, :], op=OP.subtract)
                st['X'], st['Xhi'], st['Xlo'] = X, Xhi, Xlo
                st['h'] = None
                return st

            def stage_rb(st, k):
                ci = st['ci']
                ps_r = psum(48, f"psr_{ci}_{k}")
                mm3(ps_r, WT['rb1s_hi'][:, :], WT['rb1s_lo'][:, :],
                    st['Xhi'], st['Xlo'])
                r_ = hp.tile([48, tn], f32, tag="r", name=f"r_{ci}_{k}")
                nc.scalar.activation(r_[:, :], ps_r[:, :], A.Relu,
                                     bias=WT['b_rb1s'][:, 0:1])
                ps_x = psum(96, f"psx_{ci}_{k}")
                mm(ps_x, WT['rb2s'][:, :], r_, True, True)
                sx = sp.tile([96, tn], f32, tag="sx", name=f"sx_{ci}_{k}")
                nc.vector.scalar_tensor_tensor(
                    sx[:, :], st['X'][:, :], WT['sxv96'][:, 0:1],
                    ps_x[:, :], op0=OP.mult, op1=OP.add)
                X = xp.tile([96, tn], f32, tag="x", name=f"x_{ci}_{k}")
                nc.scalar.activation(X[:, :], sx[:, :], A.Relu,
                                     bias=WT['b_rb2s'][:, 0:1])
                Xhi = xp.tile([96, tn], f32r, tag="xhi", name=f"xhi_{ci}_{k}")
                nc.scalar.activation(Xhi[:, :], sx[:, :], A.Relu,
                                     bias=WT['b_rb2s'][:, 0:1])
                Xlo = xp.tile([96, tn], f32r, tag="xlo", name=f"xlo_{ci}_{k}")
                nc.vector.tensor_tensor(out=Xlo[:, :], in0=X[:, :],
                                        in1=Xhi[:, :], op=OP.subtract)
                st['X'], st['Xhi'], st['Xlo'] = X, Xhi, Xlo

            def stage_qv(st, k):
                ci = st['ci']
                ps_q = psum(112, f"psq_{ci}_{k}")
                mm3(ps_q, WT['qs_hi'][:, :], WT['qs_lo'][:, :],
                    st['Xhi'], st['Xlo'])
                qall = up.tile([112, tn], f32, tag="qall", name=f"q_{ci}_{k}")
                nc.scalar.activation(qall[:, :], ps_q[:, :], A.Identity,
                                     bias=WT['bq_g'][:, 0:1])
                ps_v = psum(112, f"psv_{ci}_{k}")
                mm3(ps_v, WT['vs_hi'][:, :], WT['vs_lo'][:, :],
                    st['Xhi'], st['Xlo'])
                vall = up.tile([112, tn], f32, tag="vall", name=f"v_{ci}_{k}")
                nc.scalar.activation(vall[:, :], ps_v[:, :], A.Identity,
                                     bias=WT['bv_g'][:, 0:1])
                st['qall'], st['vall'] = qall, vall

            def stage_cf(st):
                ci = st['ci']
                ps_cf = psum(48, f"pscf_{ci}")
                mm3(ps_cf, WT['cfs_hi'][:, :], WT['cfs_lo'][:, :],
                    st['Xhi'], st['Xlo'])
                la0 = sp.tile([48, tn], f32, tag="la0", name=f"la0_{ci}")
                nc.scalar.activation(la0[:, :], ps_cf[:, :], A.Relu,
                                     bias=WT['b_cf'][:, 0:1])
                ps_h0 = psum(24, f"psh0_{ci}")
                mm(ps_h0, WT['e1Tk0'][:, :], la0, True, True)
                h = hp.tile([24, tn], f32r, tag="h", name=f"h0_{ci}")
                nc.scalar.activation(h[:, :], ps_h0[:, :], A.Gelu,
                                     bias=WT['b_e1'][:, 0:1])
                st['h'] = h

            def stage_kk(st, k):
                ci = st['ci']
                ts_ = []
                for p in range(2):
                    ps_kk = psum(112, f"pskk{p}_{ci}_{k}")
                    mm(ps_kk, WT[f'kf{p + 1}_hi'][:, :], st['h'],
                       True, False)
                    mm(ps_kk, WT[f'kf{p + 1}_lo'][:, :], st['h'],
                       False, True)
                    t_ = hp.tile([112, tn], f32r, tag="t",
                                 name=f"t{p}_{ci}_{k}")
                    nc.vector.scalar_tensor_tensor(
                        t_[:, :], ps_kk[:, :], WT[f'bkf{p + 1}'][:, 0:1],
                        st['qall'][:, :], op0=OP.add, op1=OP.mult)
                    ts_.append(t_)
                st['ts'] = ts_

            def stage_logits(st, k):
                ci = st['ci']
                do_max = k in max_ks
                mrows = 40 if do_max else 8
                ps_ls = psum(mrows, f"psls_{ci}_{k}")
                for p in range(2):
                    mm(ps_ls, WT[f'ones16_{p + 1}'][:, 0:mrows], st['ts'][p],
                       p == 0, p == 1 and not do_max)
                if do_max:
                    sw_sb = sp.tile([8, tn], f32, tag="swsb",
                                    name=f"swsb_{ci}_{k}")
                    nc.gpsimd.tensor_copy(out=sw_sb[:, :],
                                          in_=ps_ls[32:40, :])
                    mx1 = sp.tile([8, tn], f32r, tag="mx1",
                                  name=f"mx1_{ci}_{k}")
                    nc.vector.tensor_tensor(out=mx1[:, :],
                                            in0=ps_ls[0:8, :],
                                            in1=sw_sb[:, :], op=OP.max)
                    ps_pm = psum(8, f"pspm_{ci}_{k}")
                    mm(ps_pm, WT['perm8T'][:, :], mx1, True, True)
                    mxf = sp.tile([8, tn], f32r, tag="mxf",
                                  name=f"mxf_{ci}_{k}")
                    nc.vector.tensor_tensor(out=mxf[:, :], in0=mx1[:, :],
                                            in1=ps_pm[:, :], op=OP.max)
                    mm(ps_ls[0:8, :], WT['negI8'][:, :], mxf, False, True)
                st['ps_ls'] = ps_ls

            def stage_soft(st, k):
                ci = st['ci']
                ps_ls = st['ps_ls']
                e_ = sp.tile([8, tn], f32r, tag="e", name=f"e_{ci}_{k}")
                nc.scalar.activation(e_[:, :], ps_ls[0:8, :], A.Exp)
                ps_S = psum(2, f"psS_{ci}_{k}")
                mm(ps_S, WT['sum4T'][:, :], e_, True, True)
                rr = sp.tile([2, tn], f32r, tag="rr", name=f"rr_{ci}_{k}")
                nc.vector._custom_dve(_RF, out=rr[:, :], in0=ps_S[:, :],
                                      s0=_RC["s0"], s1=_RC["s1"],
                                      imm2=_RC["imm2"])
                ps_rbc = psum(8, f"psrbc_{ci}_{k}")
                mm(ps_rbc, WT['bc28T'][:, :], rr, True, True)
                att = sp.tile([8, tn], f32r, tag="att", name=f"att_{ci}_{k}")
                nc.vector.tensor_tensor(out=att[:, :], in0=e_[:, :],
                                        in1=ps_rbc[:, :], op=OP.mult)
                st['att'] = att

            def stage_av(st, k):
                ci = st['ci']
                us = []
                for p in range(2):
                    ps_ae = psum(112, f"psae{p}_{ci}_{k}")
                    mm(ps_ae, WT[f'attexp{p + 1}T'][:, :], st['att'],
                       True, True)
                    u_ = up.tile([112, tn], f32, tag="u",
                                 name=f"u{p}_{ci}_{k}")
                    nc.vector.tensor_tensor(out=u_[:, :], in0=ps_ae[:, :],
                                            in1=st['vall'][:, :],
                                            op=OP.mult)
                    us.append(u_)
                ps_h = psum(24, f"psh_{ci}_{k}")
                mm(ps_h, WT['e1aT'][:, :], us[0], True, False)
                mm(ps_h, WT['e1bT'][:, :], us[1], False, True)
                h = hp.tile([24, tn], f32r, tag="h", name=f"h_{ci}_{k}")
                nc.scalar.activation(h[:, :], ps_h[:, :], A.Gelu,
                                     bias=WT['b_e1'][:, 0:1])
                st['h'] = h

            def finalize(st):
                ci = st['ci']
                ps_o = psum(48, f"pso_{ci}")
                mm(ps_o, WT['e2hT_hi'][:, :], st['h'], True, False)
                mm(ps_o, WT['e2hT_lo'][:, :], st['h'], False, True)
                out = sp.tile([48, tn], f32, tag="out", name=f"out_{ci}")
                nc.scalar.activation(out[:, :], ps_o[:, :], A.Identity,
                                     bias=WT['b_e2'][:, 0:1])
                nc.sync.dma_start(
                    out=out_d[st['bimg'], :, st['off']:st['off'] + tn],
                    in_=out[:, :])

            LANES = 2
            rep_ctx = tc.For_i(0, repeat, 1) if repeat > 1 else nullcontext()
            with rep_ctx:
              for cp in range(0, nchunk, LANES):
                sts = [new_state(cp + j) for j in range(LANES)]
                for k in range(4):
                    for st in sts:
                        stage_rb(st, k)
                    for st in sts:
                        stage_qv(st, k)
                    if k == 0:
                        for st in sts:
                            stage_cf(st)
                    for st in sts:
                        stage_kk(st, k)
                    for st in sts:
                        stage_logits(st, k)
                    for st in sts:
                        stage_soft(st, k)
                    for st in sts:
                        stage_av(st, k)
                for st in sts:
                    finalize(st)
    nc.compile()
    return nc


# ---------------------------------------------------------------- entry
def kernel(**inputs):
    from concourse.bass_utils import run_bass_kernel_spmd

    key = "full"
    if key not in _prog_cache:
        _prog_cache[key] = build_program()
    nc = _prog_cache[key]

    P = fold_params({k: np.asarray(v) for k, v in inputs.items()})
    x0 = np.asarray(inputs['x0'], np.float32).reshape(B, C, HW)
    x1 = np.asarray(inputs['x1'], np.float32).reshape(B, C, HW)
    in_maps = []
    for c in range(N_CORES):
        m = dict(P)
        m['x0'] = np.ascontiguousarray(x0[c * B_LOC:(c + 1) * B_LOC])
        m['x1'] = np.ascontiguousarray(x1[c * B_LOC:(c + 1) * B_LOC])
        in_maps.append(m)
    res = run_bass_kernel_spmd(nc, in_maps, list(range(N_CORES)))
    out = np.concatenate([res.results[c]['out'] for c in range(N_CORES)], 0)
    return out.reshape(B, C, H, W).astype(np.float32)


if __name__ == '__main__':
    import reference as R
    inputs = R.setup_inputs()
    expected = np.asarray(R.reference(**inputs))
    actual = kernel(**{k: np.asarray(v) for k, v in inputs.items()})
    denom = np.abs(expected).max()
    rel = np.abs(actual - expected).max() / denom
    print('rel err:', rel)
